# revision 1
# baseline (speedup 1.0000x reference)
"""DGCN diffusion-graph-conv kernel for 8 Trainium2 NeuronCores.

Math (per the reference):
    support S = D^-1/2 (adj+I)^T D^-1/2  with D = diag(rowsum(adj+I))
    x_m = T_m(S) x0  (Chebyshev recurrence, K=3 -> m=0..3)
    out = sum_m x_m @ W_m + bias

Implementation strategy (data-parallel over batch, 4 batches/core):
    Rewrite out = sum_m T_m(S) (x0 @ W_m) and fold the Chebyshev
    coefficients into the weights:
        V0 = W0 - W2, V1 = W1 - 3*W3, V2 = 2*W2, V3 = 4*W3
        U_m = x0 @ V_m   (projection; contracts feature dim d)
        out = U0 + S*(U1 + S*(U2 + S*U3))   (Horner; contracts node dim n)
    The projection's stationary operand is x0^T, which the host supplies
    directly (layout prep during sharding).  All matmuls run in fp32r
    (fp22 multiply / fp32 accumulate) at full PE rate.
"""

import numpy as np

import concourse.bacc as bacc
import concourse.tile as tile
import concourse.mybir as mybir
from concourse.bass_utils import run_bass_kernel_spmd

F32 = mybir.dt.float32
F32R = mybir.dt.float32r
AX = mybir.AxisListType
ALU = mybir.AluOpType

N_CORES = 8
B, N, D = 32, 512, 768
BL = B // N_CORES          # local batches per core = 4
BN = BL * N                # local rows = 2048
NT = BN // 128             # 16 row tiles
DT = D // 128              # 6 feature tiles
JT = N // 128              # 4 node tiles
WE = 256                   # output-column block width
EB = D // WE               # 3 column blocks


def _build_program():
    nc = bacc.Bacc("TRN2", target_bir_lowering=False, debug=False,
                   num_devices=N_CORES)
    # x0^T for this core: [d, (b n)]
    inpT_d = nc.dram_tensor("inpT", [D, BN], F32, kind="ExternalInput").ap()
    adj_d = nc.dram_tensor("adj", [N, N], F32, kind="ExternalInput").ap()
    wts_d = nc.dram_tensor("wts", [D * 4, D], F32, kind="ExternalInput").ap()
    bias_d = nc.dram_tensor("bias", [D], F32, kind="ExternalInput").ap()
    eye_d = nc.dram_tensor("eye", [128, 128], F32, kind="ExternalInput").ap()
    out_d = nc.dram_tensor("out", [BN, D], F32, kind="ExternalOutput").ap()
    dscr = nc.dram_tensor("dscr", [N], F32)

    # weights viewed as [m, d, e] (reference row index is d*4+m)
    wts_v = wts_d.rearrange("(d m) e -> m d e", m=4)

    with tile.TileContext(nc) as tc:
        with (
            tc.tile_pool(name="const", bufs=1) as constp,
            tc.tile_pool(name="sup", bufs=1) as supp,
            tc.tile_pool(name="x0T", bufs=1) as x0Tp,
            tc.tile_pool(name="wst", bufs=12) as wp,
            tc.tile_pool(name="vt", bufs=24) as vp,
            tc.tile_pool(name="ut", bufs=25) as up,
            tc.tile_pool(name="pg", bufs=7) as pgp,
            tc.tile_pool(name="stg", bufs=4) as stgp,
            tc.tile_pool(name="ps", bufs=8, space="PSUM") as psp,
        ):
            def load_v(eb, dts=None, v=None):
                """DMA the W column block and build the V combos."""
                c0 = eb * WE
                if v is None:
                    v = [[None] * DT for _ in range(2)]
                for dt in (dts if dts is not None else range(DT)):
                    w_raw = [None] * 4
                    for m in (0, 2, 1, 3):
                        w = wp.tile([128, WE], F32,
                                    name=f"w{eb}_{dt}_{m}", tag="wt")
                        nc.sync.dma_start(
                            w[:],
                            wts_v[m, dt * 128:(dt + 1) * 128, c0:c0 + WE])
                        w_raw[m] = w[:]
                    vp01 = vp.tile([128, 2, WE], F32R,
                                   name=f"v{eb}_{dt}_01", tag="vt")
                    nc.vector.tensor_sub(vp01[:, 0, :], w_raw[0], w_raw[2])
                    nc.vector.scalar_tensor_tensor(
                        vp01[:, 1, :], w_raw[3], -3.0, w_raw[1],
                        ALU.mult, ALU.add)
                    vp23 = vp.tile([128, 2, WE], F32R,
                                   name=f"v{eb}_{dt}_23", tag="vt")
                    nc.vector.tensor_scalar_mul(vp23[:, 0, :], w_raw[2], 2.0)
                    nc.vector.tensor_scalar_mul(vp23[:, 1, :], w_raw[3], 4.0)
                    v[0][dt], v[1][dt] = vp01, vp23
                return v

            eye128 = constp.tile([128, 128], F32)
            nc.gpsimd.dma_start(eye128[:], eye_d[:])

            # ---- DMA issue order: first-needed first ----
            # x0^T chunk 0 (row tiles bt=0..3), then eb0 weights, then the
            # rest of x0^T, then support/bias inputs.
            x0T = []
            for dt in range(DT):
                t = x0Tp.tile([128, BN], F32R, name=f"x0T{dt}")
                x0T.append(t)
            adjts = []
            for t in range(JT):
                adjt = supp.tile([128, N], F32, name=f"adjt{t}")
                nc.gpsimd.dma_start(adjt[:], adj_d[t * 128:(t + 1) * 128, :])
                adjts.append(adjt)

            # interleave eb0 weights with the first x0^T chunks in the order
            # the first projection consumes them
            v_cur = None
            for dt in range(DT):
                nc.sync.dma_start(
                    x0T[dt][:, 0:256],
                    inpT_d[dt * 128:(dt + 1) * 128, 0:256].bitcast(F32R))
                v_cur = load_v(0, dts=[dt], v=v_cur)

            for dt in range(DT):
                nc.sync.dma_start(
                    x0T[dt][:, 256:512],
                    inpT_d[dt * 128:(dt + 1) * 128, 256:512].bitcast(F32R))
            for ck in range(1, 4):
                for dt in range(DT):
                    eng = nc.gpsimd if ck == 3 else nc.sync
                    eng.dma_start(
                        x0T[dt][:, ck * 512:(ck + 1) * 512],
                        inpT_d[dt * 128:(dt + 1) * 128,
                               ck * 512:(ck + 1) * 512].bitcast(F32R))

            bias_bc = constp.tile([128, D], F32)
            nc.gpsimd.dma_start(
                bias_bc[:], bias_d.unsqueeze(0).broadcast_to([128, D]))

            # ---- support matrix S^T = (adj+I) * d[j]d[i], built as
            #      adj*d[j]d[i] plus a diagonal d^2 fix-up ----
            dcols, dsqs = [], []
            for t in range(JT):
                adjt = adjts[t]
                rs = supp.tile([128, 1], F32, name=f"rs{t}", tag="rs",
                               bufs=2)
                nc.vector.tensor_reduce(rs[:], adjt[:], axis=AX.X, op=ALU.add)
                nc.vector.tensor_scalar_add(rs[:], rs[:], 1.0)
                sq = supp.tile([128, 1], F32, name=f"sq{t}", tag="sq",
                               bufs=2)
                nc.scalar.sqrt(sq[:], rs[:])
                dcol = supp.tile([128, 1], F32, name=f"dcol{t}")
                nc.vector.reciprocal(dcol[:], sq[:])
                dsq = supp.tile([128, 1], F32, name=f"dsq{t}")
                nc.vector.tensor_mul(dsq[:], dcol[:], dcol[:])
                nc.gpsimd.dma_start(dscr.ap()[t * 128:(t + 1) * 128],
                                    dcol[:])
                dcols.append(dcol)
                dsqs.append(dsq)
            dbc = constp.tile([128, N], F32)
            nc.gpsimd.dma_start(
                dbc[:], dscr.ap().unsqueeze(0).broadcast_to([128, N]))
            st_t = []
            for t in range(JT):
                s = supp.tile([128, N], F32R, name=f"st{t}")
                nc.vector.scalar_tensor_tensor(
                    s[:], adjts[t][:], dcols[t][:], dbc[:],
                    ALU.mult, ALU.mult)
                diagfix = supp.tile([128, 128], F32, name=f"dfix{t}",
                                    tag="dfix", bufs=2)
                nc.vector.tensor_scalar_mul(diagfix[:], eye128[:], dsqs[t][:])
                nc.vector.tensor_add(
                    s[:, t * 128:(t + 1) * 128],
                    s[:, t * 128:(t + 1) * 128], diagfix[:])
                st_t.append(s)

            # ---- main loops: per column-block project then Horner ----
            for eb in range(EB):
                c0 = eb * WE
                v = v_cur

                def proj(b, u=None):
                    # projection for batch b; U stored in batch-pair tiles
                    # [128, 2, WE] (dim1 = b parity) shared with b^1
                    h = b % 2
                    if u is None:
                        u = [[None] * JT for _ in range(4)]
                        for m in range(4):
                            for nt in range(JT):
                                u[m][nt] = up.tile(
                                    [128, 2, WE], F32R,
                                    name=f"u{eb}_{b // 2}_{nt}_{m}",
                                    tag="ut")
                    for nt in range(JT):
                        bt = b * JT + nt
                        for pr in range(2):
                            pmt = psp.tile([128, 2, WE], F32,
                                           name=f"pp{eb}_{bt}_{pr}",
                                           tag="ps")
                            for dt in range(DT):
                                lhs = x0T[dt][:, bt * 128:(bt + 1) * 128]
                                nc.tensor.matmul(
                                    pmt[:], lhs, v[pr][dt][:],
                                    start=(dt == 0), stop=(dt == DT - 1))
                            for half in range(2):
                                m = pr * 2 + half
                                if m == 0:
                                    nc.vector.tensor_add(
                                        u[m][nt][:, h, :], pmt[:, 0, :],
                                        bias_bc[:, c0:c0 + WE])
                                else:
                                    nc.scalar.copy(
                                        u[m][nt][:, h, :], pmt[:, half, :])
                    return u

                def horner(bp, u):
                    # Horner for batch pair bp (b = 2*bp, 2*bp+1), N=512
                    # matmuls over the pair dim.  P2 -> fresh tiles (u[3] is
                    # still read by later-traced matmuls), P1 -> u[3],
                    # out -> staged + one strided DMA per nt
                    src_t = u[3]
                    for step, (madd, dest) in enumerate(
                            [(2, "fresh"), (1, 3), (0, None)]):
                        new_t = [None] * JT
                        for nt in range(JT):
                            ph = psp.tile([128, 2, WE], F32,
                                          name=f"phh{eb}_{bp}_{step}_{nt}",
                                          tag="ps")
                            for jt in range(JT):
                                nc.tensor.matmul(
                                    ph[:],
                                    st_t[jt][:, nt * 128:(nt + 1) * 128],
                                    src_t[jt][:],
                                    start=(jt == 0), stop=(jt == JT - 1))
                            if dest == "fresh":
                                pgt = pgp.tile([128, 2, WE], F32R,
                                               name=f"pg{eb}_{bp}_{nt}",
                                               tag="pg")
                                nc.vector.tensor_add(
                                    pgt[:], ph[:], u[madd][nt][:])
                                new_t[nt] = pgt
                            elif dest is not None:
                                nc.vector.tensor_add(
                                    u[dest][nt][:], ph[:], u[madd][nt][:])
                                new_t[nt] = u[dest][nt]
                            else:
                                so = stgp.tile([128, 2, WE], F32,
                                               name=f"so{eb}_{bp}_{nt}",
                                               tag="outst")
                                nc.vector.tensor_add(
                                    so[:], ph[:], u[0][nt][:])
                                r0 = (2 * bp * JT + nt) * 128
                                nc.sync.dma_start(
                                    out_d.rearrange(
                                        "(x p) e -> p x e", p=128)[
                                        :, r0 // 128:r0 // 128 + 5:4,
                                        c0:c0 + WE],
                                    so[:])
                        src_t = new_t

                # software pipeline: keep independent projection work
                # available while each Horner chain waits on evictions
                u0p = proj(0)
                u0p = proj(1, u0p)
                if eb + 1 < EB:
                    v_next = load_v(eb + 1)
                u1p = proj(2)
                horner(0, u0p)
                u1p = proj(3, u1p)
                horner(1, u1p)
                if eb + 1 < EB:
                    v_cur = v_next
    nc.compile()
    return nc


_CACHE = {}


def _get_program():
    if "nc" not in _CACHE:
        _CACHE["nc"] = _build_program()
    return _CACHE["nc"]


def make_in_maps(inputs, adj, weights, biases):
    inputs = np.ascontiguousarray(inputs, dtype=np.float32)
    adj = np.ascontiguousarray(adj, dtype=np.float32)
    weights = np.ascontiguousarray(weights, dtype=np.float32)
    biases = np.ascontiguousarray(biases, dtype=np.float32)
    assert inputs.shape == (B, N, D)
    assert adj.shape == (N, N)
    assert weights.shape == (D * 4, D)
    assert biases.shape == (D,)
    eye = np.eye(128, dtype=np.float32)
    in_maps = []
    for c in range(N_CORES):
        x0T = np.ascontiguousarray(
            inputs[c * BL:(c + 1) * BL].reshape(BN, D).T)
        in_maps.append({
            "inpT": x0T,
            "adj": adj,
            "wts": weights,
            "bias": biases,
            "eye": eye,
        })
    return in_maps


def kernel(inputs, adj, weights, biases):
    nc = _get_program()
    in_maps = make_in_maps(inputs, adj, weights, biases)
    res = run_bass_kernel_spmd(nc, in_maps, list(range(N_CORES)))
    out = np.concatenate(
        [res.results[c]["out"].reshape(BL, N, D) for c in range(N_CORES)],
        axis=0)
    return out



# revision 2
# speedup vs baseline: 1.3841x; 1.3841x over previous
"""DGCN diffusion-graph-conv kernel for 8 Trainium2 NeuronCores.

Math (per the reference):
    support S = D^-1/2 (adj+I)^T D^-1/2  with D = diag(rowsum(adj+I))
    x_m = T_m(S) x0  (Chebyshev recurrence, K=3 -> m=0..3)
    out = sum_m x_m @ W_m + bias

Strategy (data-parallel over batch, 4 batches/core):
    Fold Chebyshev coefficients into the weights:
        V0 = W0 - W2, V1 = W1 - 3*W3, V2 = 2*W2, V3 = 4*W3
        U_m = x0 @ V_m
        out = U0 + S U1 + S^2 U2 + S^3 U3
    Precision split: the m=0 term dominates the output magnitude and is
    computed in bf16; the m=1..3 terms are attenuated ~20x by each S
    application, so they run in fp8 (e4m3) with DoubleRow matmuls at 2x
    PE throughput.  S, S^2, S^3 are built on device in fp8 (x128 scale),
    making the three diffusion applications one independent PSUM
    accumulation per output tile (no serial Horner chain).
"""

import numpy as np
import ml_dtypes

import concourse.bacc as bacc
import concourse.tile as tile
import concourse.mybir as mybir
from concourse.bass_utils import run_bass_kernel_spmd

F32 = mybir.dt.float32
BF16 = mybir.dt.bfloat16
FP8 = mybir.dt.float8e4
AX = mybir.AxisListType
ALU = mybir.AluOpType
DR = mybir.MatmulPerfMode.DoubleRow

N_CORES = 8
B, N, D = 32, 512, 768
BL = B // N_CORES          # local batches per core = 4
BN = BL * N                # local rows = 2048
NT = BN // 128             # 16 row tiles
JT = N // 128              # 4 node tiles
WE = 256                   # output-column block width
EB = D // WE               # 3 column blocks
GD = D // 256              # 3 d-groups of 256 for DoubleRow contraction
S_SC = 128.0               # fp8 scale on the S-power chain (2^7)
V_SC = 32.0                # fp8 scale on V1..V3 (2^5)


def _build_program():
    nc = bacc.Bacc("TRN2", target_bir_lowering=False, debug=False,
                   num_devices=N_CORES)
    x8_d = nc.dram_tensor("x8", [GD, 128, 2, BN], FP8,
                          kind="ExternalInput").ap()
    xbf_d = nc.dram_tensor("xbf", [D, BN], BF16, kind="ExternalInput").ap()
    v8_d = nc.dram_tensor("v8", [GD, 128, 2, 3 * D], FP8,
                          kind="ExternalInput").ap()
    v0b_d = nc.dram_tensor("v0b", [D, D], BF16, kind="ExternalInput").ap()
    adj_d = nc.dram_tensor("adj", [N, N], F32, kind="ExternalInput").ap()
    adjt_d = nc.dram_tensor("adjt", [N, N], F32, kind="ExternalInput").ap()
    bias_d = nc.dram_tensor("bias", [D], F32, kind="ExternalInput").ap()
    eye_d = nc.dram_tensor("eye", [128, 128], F32, kind="ExternalInput").ap()
    out_d = nc.dram_tensor("out", [BN, D], F32, kind="ExternalOutput").ap()
    dscr = nc.dram_tensor("dscr", [N], F32)

    with tile.TileContext(nc) as tc:
        with (
            tc.tile_pool(name="const", bufs=1) as constp,
            tc.tile_pool(name="adjp", bufs=1) as adjp,
            tc.tile_pool(name="xp", bufs=1) as xp,
            tc.tile_pool(name="vp", bufs=1) as vp,
            tc.tile_pool(name="sp", bufs=1) as sp,
            tc.tile_pool(name="s8p", bufs=1) as s8p,
            tc.tile_pool(name="u0p", bufs=1) as u0p,
            tc.tile_pool(name="u8p", bufs=1) as u8p,
            tc.tile_pool(name="stg", bufs=6) as stgp,
            tc.tile_pool(name="ps", bufs=8, space="PSUM") as psp,
        ):
            # ---- input DMAs ----
            # gpsimd queue: adj/adjT (S chain needs them early), then the
            # bf16 m=0 operands (first consumed ~25us in).
            # sync queue: the fp8 projection operands (first consumed
            # ~10us in, right after the S-power builds).
            eye128 = constp.tile([128, 128], F32)
            nc.gpsimd.dma_start(eye128[:], eye_d[:])
            adjts, adjTts = [], []
            for t in range(JT):
                a = adjp.tile([128, N], F32, name=f"adjt{t}")
                nc.gpsimd.dma_start(a[:], adj_d[t * 128:(t + 1) * 128, :])
                adjts.append(a)
            for t in range(JT):
                a = adjp.tile([128, N], F32, name=f"adjTt{t}")
                nc.gpsimd.dma_start(a[:], adjt_d[t * 128:(t + 1) * 128, :])
                adjTts.append(a)
            bias_bc = constp.tile([128, D], F32)
            nc.gpsimd.dma_start(
                bias_bc[:], bias_d.unsqueeze(0).broadcast_to([128, D]))

            x8t = []
            for g in range(GD):
                t8 = xp.tile([128, 2, BN], FP8, name=f"x8t{g}")
                nc.sync.dma_start(t8[:], x8_d[g])
                x8t.append(t8)
            v8t = []
            for g in range(GD):
                t8 = vp.tile([128, 2, 3 * D], FP8, name=f"v8t{g}")
                nc.sync.dma_start(t8[:], v8_d[g])
                v8t.append(t8)
            xbf = []
            for dt in range(D // 128):
                t = xp.tile([128, BN], BF16, name=f"xbf{dt}")
                nc.gpsimd.dma_start(t[:], xbf_d[dt * 128:(dt + 1) * 128, :])
                xbf.append(t)
            v0bt = []
            for dt in range(D // 128):
                t = vp.tile([128, D], BF16, name=f"v0bt{dt}")
                nc.gpsimd.dma_start(t[:], v0b_d[dt * 128:(dt + 1) * 128, :])
                v0bt.append(t)

            # ---- support matrix:  S^T[i,j] = adj[i,j] d_i d_j + delta d^2
            #      and            S[i,j] = adjT[i,j] d_i d_j + delta d^2 ----
            dcols, dsqs = [], []
            for t in range(JT):
                rs = sp.tile([128, 1], F32, name=f"rs{t}", tag="rs", bufs=2)
                nc.vector.tensor_reduce(rs[:], adjts[t][:], axis=AX.X,
                                        op=ALU.add)
                nc.vector.tensor_scalar_add(rs[:], rs[:], 1.0)
                sq = sp.tile([128, 1], F32, name=f"sq{t}", tag="sq", bufs=2)
                nc.scalar.sqrt(sq[:], rs[:])
                dcol = sp.tile([128, 1], F32, name=f"dcol{t}")
                nc.vector.reciprocal(dcol[:], sq[:])
                dsq = sp.tile([128, 1], F32, name=f"dsq{t}")
                nc.vector.tensor_mul(dsq[:], dcol[:], dcol[:])
                nc.gpsimd.dma_start(dscr.ap()[t * 128:(t + 1) * 128], dcol[:])
                dcols.append(dcol)
                dsqs.append(dsq)
            dbc = constp.tile([128, N], F32)
            nc.gpsimd.dma_start(
                dbc[:], dscr.ap().unsqueeze(0).broadcast_to([128, N]))

            # fp8 S-power tiles: P8[g][p, i, n] = (S^m)^T[g*256+i*128+p, n]
            s8 = [s8p.tile([128, 2, N], FP8, name=f"s8_{g}")
                  for g in range(2)]
            st8 = [s8p.tile([128, 2, N], FP8, name=f"st8_{g}")
                   for g in range(2)]
            t28 = [s8p.tile([128, 2, N], FP8, name=f"t28_{g}")
                   for g in range(2)]
            t38 = [s8p.tile([128, 2, N], FP8, name=f"t38_{g}")
                   for g in range(2)]
            for srcts, dsts in ((adjts, s8), (adjTts, st8)):
                for t in range(JT):
                    sf = sp.tile([128, N], F32, name=f"sf{t}", tag="sf",
                                 bufs=4)
                    nc.vector.scalar_tensor_tensor(
                        sf[:], srcts[t][:], dcols[t][:], dbc[:],
                        ALU.mult, ALU.mult)
                    dfix = sp.tile([128, 128], F32, name=f"dfix{t}",
                                   tag="dfix", bufs=2)
                    nc.vector.tensor_scalar_mul(dfix[:], eye128[:],
                                                dsqs[t][:])
                    nc.vector.tensor_add(
                        sf[:, t * 128:(t + 1) * 128],
                        sf[:, t * 128:(t + 1) * 128], dfix[:])
                    nc.scalar.mul(dsts[t // 2][:, t % 2, :], sf[:], S_SC)

            # ---- S^2 / S^3 in fp8 (x128 scale) via DoubleRow matmuls ----
            for rhs_t, dst in ((s8, t28), (t28, t38)):
                for jt in range(JT):
                    pst = psp.tile([128, 2, WE], F32, name=f"pst{jt}",
                                   tag="ps")
                    for g in range(2):
                        nc.tensor.matmul(
                            pst[:],
                            st8[g][:, :, jt * 128:(jt + 1) * 128],
                            rhs_t[g][:],
                            start=(g == 0), stop=(g == 1), perf_mode=DR)
                    nc.scalar.mul(dst[jt // 2][:, jt % 2, :], pst[:],
                                  1.0 / S_SC)

            # ---- per column-block projection + diffusion-apply ----
            u8tiles = {}
            u0tiles = {}

            def proj_m12m3(eb):
                c0 = eb * 3 * WE
                for m in (1, 2, 3):
                    for g2 in range(2):
                        for bp in range(2):
                            u8tiles[(eb, m, g2, bp)] = u8p.tile(
                                [128, 2, 2, WE], FP8,
                                name=f"u8_{eb}_{m}_{g2}_{bp}",
                                tag="u8", bufs=24)
                for nt in range(NT):
                    b, jt = nt // JT, nt % JT
                    g2, i2, bp, h = jt // 2, jt % 2, b // 2, b % 2
                    ps12 = psp.tile([128, 2, WE], F32,
                                    name=f"ps12_{eb}_{nt}", tag="ps")
                    for g in range(GD):
                        nc.tensor.matmul(
                            ps12[:],
                            x8t[g][:, :, nt * 128:(nt + 1) * 128],
                            v8t[g][:, :, c0:c0 + 2 * WE],
                            start=(g == 0), stop=(g == GD - 1), perf_mode=DR)
                    ps3 = psp.tile([128, 2, WE], F32,
                                   name=f"ps3_{eb}_{nt}", tag="ps")
                    for g in range(GD):
                        nc.tensor.matmul(
                            ps3[:, 0, :],
                            x8t[g][:, :, nt * 128:(nt + 1) * 128],
                            v8t[g][:, :, c0 + 2 * WE:c0 + 3 * WE],
                            start=(g == 0), stop=(g == GD - 1), perf_mode=DR)
                    nc.vector.tensor_scalar_mul(
                        u8tiles[(eb, 1, g2, bp)][:, i2, h, :],
                        ps12[:, 0, :], 1.0 / V_SC)
                    nc.vector.tensor_scalar_mul(
                        u8tiles[(eb, 2, g2, bp)][:, i2, h, :],
                        ps12[:, 1, :], 1.0 / V_SC)
                    nc.scalar.mul(
                        u8tiles[(eb, 3, g2, bp)][:, i2, h, :],
                        ps3[:, 0, :], 1.0 / V_SC)

            def proj_m0(eb):
                for bp in range(2):
                    for jt in range(JT):
                        u0tiles[(eb, bp, jt)] = u0p.tile(
                            [128, 2, WE], F32, name=f"u0_{eb}_{bp}_{jt}",
                            tag="u0", bufs=16)
                for nt in range(NT):
                    b, jt = nt // JT, nt % JT
                    bp, h = b // 2, b % 2
                    ps0 = psp.tile([128, 2, WE], F32,
                                   name=f"ps0_{eb}_{nt}", tag="ps")
                    for dt in range(D // 128):
                        nc.tensor.matmul(
                            ps0[:, 0, :],
                            xbf[dt][:, nt * 128:(nt + 1) * 128],
                            v0bt[dt][:, eb * WE:(eb + 1) * WE],
                            start=(dt == 0), stop=(dt == D // 128 - 1))
                    nc.vector.tensor_add(
                        u0tiles[(eb, bp, jt)][:, h, :], ps0[:, 0, :],
                        bias_bc[:, eb * WE:(eb + 1) * WE])

            def apply_(eb):
                for bp in range(2):
                    for jt in range(JT):
                        ph = psp.tile([128, 2, WE], F32,
                                      name=f"ph_{eb}_{bp}_{jt}", tag="ps")
                        k = 0
                        for m, pw in ((1, s8), (2, t28), (3, t38)):
                            for g in range(2):
                                nc.tensor.matmul(
                                    ph[:],
                                    pw[g][:, :, jt * 128:(jt + 1) * 128],
                                    u8tiles[(eb, m, g, bp)][:],
                                    start=(k == 0), stop=(k == 5),
                                    perf_mode=DR)
                                k += 1
                        so = stgp.tile([128, 2, WE], F32,
                                       name=f"so_{eb}_{bp}_{jt}",
                                       tag="outst")
                        nc.vector.scalar_tensor_tensor(
                            so[:], ph[:], 1.0 / S_SC,
                            u0tiles[(eb, bp, jt)][:], ALU.mult, ALU.add)
                        r0 = (2 * bp * JT + jt) * 128
                        nc.sync.dma_start(
                            out_d.rearrange("(x p) e -> p x e", p=128)[
                                :, r0 // 128:r0 // 128 + 5:4,
                                eb * WE:(eb + 1) * WE],
                            so[:])

            # software pipeline: apply(eb) is emitted after proj(eb+1) has
            # filled the PE queue, so the PE never waits on the fp8 casts
            proj_m12m3(0)
            proj_m0(0)
            proj_m12m3(1)
            apply_(0)
            proj_m0(1)
            proj_m12m3(2)
            apply_(1)
            proj_m0(2)
            apply_(2)
    nc.compile()
    return nc


_CACHE = {}


def _get_program():
    if "nc" not in _CACHE:
        _CACHE["nc"] = _build_program()
    return _CACHE["nc"]


def _q8(x):
    return np.clip(x, -240.0, 240.0).astype(ml_dtypes.float8_e4m3)


def make_in_maps(inputs, adj, weights, biases):
    inputs = np.ascontiguousarray(inputs, dtype=np.float32)
    adj = np.ascontiguousarray(adj, dtype=np.float32)
    weights = np.ascontiguousarray(weights, dtype=np.float32)
    biases = np.ascontiguousarray(biases, dtype=np.float32)
    assert inputs.shape == (B, N, D)
    assert adj.shape == (N, N)
    assert weights.shape == (D * 4, D)
    assert biases.shape == (D,)

    wv = weights.reshape(D, 4, D)
    v0 = wv[:, 0] - wv[:, 2]
    v1 = wv[:, 1] - 3.0 * wv[:, 3]
    v2 = 2.0 * wv[:, 2]
    v3 = 4.0 * wv[:, 3]
    # v8 column packing: col = eb*768 + (m-1)*256 + e
    vc = np.empty((D, 3 * D), dtype=np.float32)
    for eb in range(EB):
        for mi, vm in enumerate((v1, v2, v3)):
            vc[:, eb * 3 * WE + mi * WE:(eb * 3 * WE) + (mi + 1) * WE] = \
                vm[:, eb * WE:(eb + 1) * WE]
    v8 = _q8((vc * V_SC).reshape(GD, 2, 128, 3 * D).transpose(0, 2, 1, 3))
    v8 = np.ascontiguousarray(v8)
    v0b = np.ascontiguousarray(v0.astype(ml_dtypes.bfloat16))
    adjT = np.ascontiguousarray(adj.T)
    eye = np.eye(128, dtype=np.float32)

    in_maps = []
    for c in range(N_CORES):
        x0T = inputs[c * BL:(c + 1) * BL].reshape(BN, D).T  # [D, BN]
        x8 = _q8(x0T.reshape(GD, 2, 128, BN).transpose(0, 2, 1, 3))
        in_maps.append({
            "x8": np.ascontiguousarray(x8),
            "xbf": np.ascontiguousarray(x0T.astype(ml_dtypes.bfloat16)),
            "v8": v8,
            "v0b": v0b,
            "adj": adj,
            "adjt": adjT,
            "bias": biases,
            "eye": eye,
        })
    return in_maps


def kernel(inputs, adj, weights, biases):
    nc = _get_program()
    in_maps = make_in_maps(inputs, adj, weights, biases)
    res = run_bass_kernel_spmd(nc, in_maps, list(range(N_CORES)))
    out = np.concatenate(
        [res.results[c]["out"].reshape(BL, N, D) for c in range(N_CORES)],
        axis=0)
    return out


# revision 3
# speedup vs baseline: 1.5830x; 1.1437x over previous
"""DGCN diffusion-graph-conv kernel for 8 Trainium2 NeuronCores.

Math (per the reference):
    support S = D^-1/2 (adj+I)^T D^-1/2  with D = diag(rowsum(adj+I))
    x_m = T_m(S) x0  (Chebyshev recurrence, K=3 -> m=0..3)
    out = sum_m x_m @ W_m + bias

Strategy (data-parallel over batch, 4 batches/core):
    Fold Chebyshev coefficients into the weights:
        V0 = W0 - W2, V1 = W1 - 3*W3, V2 = 2*W2, V3 = 4*W3
        U_m = x0 @ V_m
        out = U0 + S U1 + S^2 U2 + S^3 U3
    Precision split: the m=0 term dominates the output magnitude and is
    computed in bf16; the m=1..3 terms are attenuated ~20x by each S
    application, so they run in fp8 (e4m3) with DoubleRow matmuls at 2x
    PE throughput.  S, S^2, S^3 are built on device in fp8 (x128 scale),
    making the three diffusion applications one independent PSUM
    accumulation per output tile (no serial Horner chain).
    Inputs stream on all three DMA queues (sync/gpsimd/scalar) in
    first-needed order; the PE phase order delays everything that
    depends on late operands (bf16 x0/V0, the S chain).
"""

import numpy as np
import ml_dtypes

import concourse.bacc as bacc
import concourse.tile as tile
import concourse.mybir as mybir
from concourse.bass_utils import run_bass_kernel_spmd

F32 = mybir.dt.float32
BF16 = mybir.dt.bfloat16
FP8 = mybir.dt.float8e4
AX = mybir.AxisListType
ALU = mybir.AluOpType
DR = mybir.MatmulPerfMode.DoubleRow

N_CORES = 8
B, N, D = 32, 512, 768
BL = B // N_CORES          # local batches per core = 4
BN = BL * N                # local rows = 2048
NT = BN // 128             # 16 row tiles
JT = N // 128              # 4 node tiles
WE = 256                   # output-column block width
EB = D // WE               # 3 column blocks
GD = D // 256              # 3 d-groups of 256 for DoubleRow contraction
S_SC = 128.0               # fp8 scale on the S-power chain (2^7)
V_SC = 32.0                # fp8 scale on V1..V3 (2^5)


def _build_program():
    nc = bacc.Bacc("TRN2", target_bir_lowering=False, debug=False,
                   num_devices=N_CORES)
    x8_d = nc.dram_tensor("x8", [GD, 128, 2, BN], FP8,
                          kind="ExternalInput").ap()
    xbf_d = nc.dram_tensor("xbf", [D, BN], BF16, kind="ExternalInput").ap()
    v8_d = nc.dram_tensor("v8", [GD, 128, 2, 3 * D], FP8,
                          kind="ExternalInput").ap()
    v0b_d = nc.dram_tensor("v0b", [D, D], BF16, kind="ExternalInput").ap()
    adj_d = nc.dram_tensor("adj", [N, N], F32, kind="ExternalInput").ap()
    adjt_d = nc.dram_tensor("adjt", [N, N], F32, kind="ExternalInput").ap()
    bias_d = nc.dram_tensor("bias", [D], F32, kind="ExternalInput").ap()
    eye_d = nc.dram_tensor("eye", [128, 128], F32, kind="ExternalInput").ap()
    out_d = nc.dram_tensor("out", [BN, D], F32, kind="ExternalOutput").ap()
    dscr = nc.dram_tensor("dscr", [N], F32)

    with tile.TileContext(nc) as tc:
        with (
            tc.tile_pool(name="const", bufs=1) as constp,
            tc.tile_pool(name="adjp", bufs=1) as adjp,
            tc.tile_pool(name="xp", bufs=1) as xp,
            tc.tile_pool(name="vp", bufs=1) as vp,
            tc.tile_pool(name="sp", bufs=1) as sp,
            tc.tile_pool(name="s8p", bufs=1) as s8p,
            tc.tile_pool(name="u0p", bufs=1) as u0p,
            tc.tile_pool(name="u8p", bufs=1) as u8p,
            tc.tile_pool(name="stg", bufs=6) as stgp,
            tc.tile_pool(name="ps", bufs=8, space="PSUM") as psp,
        ):
            # ---- input DMAs, three queues, first-needed first ----
            # The first projection phase needs all of x8 plus v8's eb0
            # columns, so those split across the three queues; the S
            # chain inputs ride gpsimd behind its x8 share; the bf16
            # m=0 operands ride scalar (consumed ~40us in).
            qs = [nc.sync, nc.gpsimd, nc.scalar]
            x8t, v8t = [], []
            for g in range(GD):
                t8 = xp.tile([128, 2, BN], FP8, name=f"x8t{g}")
                qs[g].dma_start(t8[:], x8_d[g])
                x8t.append(t8)
            for g in range(GD):
                v8t.append(vp.tile([128, 2, 3 * D], FP8, name=f"v8t{g}"))
            for g in range(GD):
                qs[g].dma_start(v8t[g][:, :, 0:3 * WE],
                                v8_d[g][:, :, 0:3 * WE])
            for eb in range(1, EB):
                for g in range(GD):
                    nc.sync.dma_start(
                        v8t[g][:, :, eb * 3 * WE:(eb + 1) * 3 * WE],
                        v8_d[g][:, :, eb * 3 * WE:(eb + 1) * 3 * WE])

            adjts, adjTts = [], []
            for t in range(JT):
                a = adjp.tile([128, N], F32, name=f"adjt{t}")
                nc.gpsimd.dma_start(a[:], adj_d[t * 128:(t + 1) * 128, :])
                adjts.append(a)
            for t in range(JT):
                a = adjp.tile([128, N], F32, name=f"adjTt{t}")
                nc.gpsimd.dma_start(a[:], adjt_d[t * 128:(t + 1) * 128, :])
                adjTts.append(a)
            eye128 = constp.tile([128, 128], F32)
            nc.gpsimd.dma_start(eye128[:], eye_d[:])
            bias_bc = constp.tile([128, D], F32)
            nc.gpsimd.dma_start(
                bias_bc[:], bias_d.unsqueeze(0).broadcast_to([128, D]))

            v0bt = []
            for dt in range(D // 128):
                t = vp.tile([128, D], BF16, name=f"v0bt{dt}")
                nc.scalar.dma_start(t[:], v0b_d[dt * 128:(dt + 1) * 128, :])
                v0bt.append(t)
            xbf = []
            for dt in range(D // 128):
                t = xp.tile([128, BN], BF16, name=f"xbf{dt}")
                nc.scalar.dma_start(t[:], xbf_d[dt * 128:(dt + 1) * 128, :])
                xbf.append(t)

            # ---- support matrix:  S^T[i,j] = adj[i,j] d_i d_j + delta d^2
            #      and              S[i,j] = adjT[i,j] d_i d_j + delta d^2 ----
            dcols, dsqs = [], []
            for t in range(JT):
                rs = sp.tile([128, 1], F32, name=f"rs{t}", tag="rs", bufs=2)
                nc.vector.tensor_reduce(rs[:], adjts[t][:], axis=AX.X,
                                        op=ALU.add)
                nc.vector.tensor_scalar_add(rs[:], rs[:], 1.0)
                sq = sp.tile([128, 1], F32, name=f"sq{t}", tag="sq", bufs=2)
                nc.scalar.sqrt(sq[:], rs[:])
                dcol = sp.tile([128, 1], F32, name=f"dcol{t}")
                nc.vector.reciprocal(dcol[:], sq[:])
                dsq = sp.tile([128, 1], F32, name=f"dsq{t}")
                nc.vector.tensor_mul(dsq[:], dcol[:], dcol[:])
                nc.gpsimd.dma_start(dscr.ap()[t * 128:(t + 1) * 128], dcol[:])
                dcols.append(dcol)
                dsqs.append(dsq)
            dbc = constp.tile([128, N], F32)
            nc.gpsimd.dma_start(
                dbc[:], dscr.ap().unsqueeze(0).broadcast_to([128, N]))

            # fp8 S-power tiles: P8[g][p, i, n] = (S^m)^T[g*256+i*128+p, n]
            s8 = [s8p.tile([128, 2, N], FP8, name=f"s8_{g}")
                  for g in range(2)]
            st8 = [s8p.tile([128, 2, N], FP8, name=f"st8_{g}")
                   for g in range(2)]
            t28 = [s8p.tile([128, 2, N], FP8, name=f"t28_{g}")
                   for g in range(2)]
            t38 = [s8p.tile([128, 2, N], FP8, name=f"t38_{g}")
                   for g in range(2)]
            for srcts, dsts in ((adjts, s8), (adjTts, st8)):
                for t in range(JT):
                    sf = sp.tile([128, N], F32, name=f"sf{t}", tag="sf",
                                 bufs=4)
                    nc.vector.scalar_tensor_tensor(
                        sf[:], srcts[t][:], dcols[t][:], dbc[:],
                        ALU.mult, ALU.mult)
                    dfix = sp.tile([128, 128], F32, name=f"dfix{t}",
                                   tag="dfix", bufs=2)
                    nc.vector.tensor_scalar_mul(dfix[:], eye128[:],
                                                dsqs[t][:])
                    nc.vector.tensor_add(
                        sf[:, t * 128:(t + 1) * 128],
                        sf[:, t * 128:(t + 1) * 128], dfix[:])
                    nc.scalar.mul(dsts[t // 2][:, t % 2, :], sf[:], S_SC)

            def powers():
                # S^2 / S^3 in fp8 (x128 scale) via DoubleRow matmuls
                for rhs_t, dst in ((s8, t28), (t28, t38)):
                    for jt in range(JT):
                        pst = psp.tile([128, 2, WE], F32,
                                       name=f"pst{jt}", tag="ps")
                        for g in range(2):
                            nc.tensor.matmul(
                                pst[:],
                                st8[g][:, :, jt * 128:(jt + 1) * 128],
                                rhs_t[g][:],
                                start=(g == 0), stop=(g == 1), perf_mode=DR)
                        nc.scalar.mul(dst[jt // 2][:, jt % 2, :], pst[:],
                                      1.0 / S_SC)

            # ---- per column-block projection + diffusion-apply ----
            u12tiles = {}
            u3tiles = {}
            u0tiles = {}

            def proj_m12m3(eb):
                c0 = eb * 3 * WE
                for g2 in range(2):
                    for bp in range(2):
                        u12tiles[(eb, g2, bp)] = u8p.tile(
                            [128, 2, 2, 2, WE], FP8,
                            name=f"u12_{eb}_{g2}_{bp}", tag="u12", bufs=8)
                        u3tiles[(eb, g2, bp)] = u8p.tile(
                            [128, 2, 2, WE], FP8,
                            name=f"u3_{eb}_{g2}_{bp}", tag="u3", bufs=8)
                for nt in range(NT):
                    b, jt = nt // JT, nt % JT
                    g2, i2, bp, h = jt // 2, jt % 2, b // 2, b % 2
                    ps12 = psp.tile([128, 2, WE], F32,
                                    name=f"ps12_{eb}_{nt}", tag="ps")
                    for g in range(GD):
                        nc.tensor.matmul(
                            ps12[:],
                            x8t[g][:, :, nt * 128:(nt + 1) * 128],
                            v8t[g][:, :, c0:c0 + 2 * WE],
                            start=(g == 0), stop=(g == GD - 1), perf_mode=DR)
                    ps3 = psp.tile([128, 2, WE], F32,
                                   name=f"ps3_{eb}_{nt}", tag="ps")
                    for g in range(GD):
                        nc.tensor.matmul(
                            ps3[:, 0, :],
                            x8t[g][:, :, nt * 128:(nt + 1) * 128],
                            v8t[g][:, :, c0 + 2 * WE:c0 + 3 * WE],
                            start=(g == 0), stop=(g == GD - 1), perf_mode=DR)
                    nc.vector.tensor_scalar_mul(
                        u12tiles[(eb, g2, bp)][:, i2, :, h, :],
                        ps12[:], 1.0 / V_SC)
                    nc.scalar.mul(
                        u3tiles[(eb, g2, bp)][:, i2, h, :],
                        ps3[:, 0, :], 1.0 / V_SC)

            def proj_m0(eb):
                for bp in range(2):
                    for jt in range(JT):
                        u0tiles[(eb, bp, jt)] = u0p.tile(
                            [128, 2, WE], F32, name=f"u0_{eb}_{bp}_{jt}",
                            tag="u0", bufs=16)
                for nt in range(NT):
                    b, jt = nt // JT, nt % JT
                    bp, h = b // 2, b % 2
                    ps0 = psp.tile([128, 2, WE], F32,
                                   name=f"ps0_{eb}_{nt}", tag="ps")
                    for dt in range(D // 128):
                        nc.tensor.matmul(
                            ps0[:, 0, :],
                            xbf[dt][:, nt * 128:(nt + 1) * 128],
                            v0bt[dt][:, eb * WE:(eb + 1) * WE],
                            start=(dt == 0), stop=(dt == D // 128 - 1))
                    nc.vector.tensor_add(
                        u0tiles[(eb, bp, jt)][:, h, :], ps0[:, 0, :],
                        bias_bc[:, eb * WE:(eb + 1) * WE])

            def apply_(eb):
                for bp in range(2):
                    for jt in range(JT):
                        ph = psp.tile([128, 2, WE], F32,
                                      name=f"ph_{eb}_{bp}_{jt}", tag="ps")
                        k = 0
                        for mi, pw in ((0, s8), (1, t28), (None, t38)):
                            for g in range(2):
                                if mi is None:
                                    rhs = u3tiles[(eb, g, bp)][:]
                                else:
                                    rhs = u12tiles[(eb, g, bp)][:, :, mi, :, :]
                                nc.tensor.matmul(
                                    ph[:],
                                    pw[g][:, :, jt * 128:(jt + 1) * 128],
                                    rhs,
                                    start=(k == 0), stop=(k == 5),
                                    perf_mode=DR)
                                k += 1
                        so = stgp.tile([128, 2, WE], F32,
                                       name=f"so_{eb}_{bp}_{jt}",
                                       tag="outst")
                        nc.vector.scalar_tensor_tensor(
                            so[:], ph[:], 1.0 / S_SC,
                            u0tiles[(eb, bp, jt)][:], ALU.mult, ALU.add)
                        r0 = (2 * bp * JT + jt) * 128
                        nc.sync.dma_start(
                            out_d.rearrange("(x p) e -> p x e", p=128)[
                                :, r0 // 128:r0 // 128 + 5:4,
                                eb * WE:(eb + 1) * WE],
                            so[:])

            # PE phase order: the fp8 projections lead (their operands
            # land first), the S-power builds and bf16 m=0 phases slot
            # in once their inputs arrive, applies trail their eb's
            # casts by a full phase.
            proj_m12m3(0)
            proj_m12m3(1)
            powers()
            proj_m0(0)
            apply_(0)
            proj_m12m3(2)
            proj_m0(1)
            apply_(1)
            proj_m0(2)
            apply_(2)
    nc.compile()
    return nc


_CACHE = {}


def _get_program():
    if "nc" not in _CACHE:
        _CACHE["nc"] = _build_program()
    return _CACHE["nc"]


def _q8(x):
    return np.clip(x, -240.0, 240.0).astype(ml_dtypes.float8_e4m3)


def make_in_maps(inputs, adj, weights, biases):
    inputs = np.ascontiguousarray(inputs, dtype=np.float32)
    adj = np.ascontiguousarray(adj, dtype=np.float32)
    weights = np.ascontiguousarray(weights, dtype=np.float32)
    biases = np.ascontiguousarray(biases, dtype=np.float32)
    assert inputs.shape == (B, N, D)
    assert adj.shape == (N, N)
    assert weights.shape == (D * 4, D)
    assert biases.shape == (D,)

    wv = weights.reshape(D, 4, D)
    v0 = wv[:, 0] - wv[:, 2]
    v1 = wv[:, 1] - 3.0 * wv[:, 3]
    v2 = 2.0 * wv[:, 2]
    v3 = 4.0 * wv[:, 3]
    # v8 column packing: col = eb*768 + (m-1)*256 + e
    vc = np.empty((D, 3 * D), dtype=np.float32)
    for eb in range(EB):
        for mi, vm in enumerate((v1, v2, v3)):
            vc[:, eb * 3 * WE + mi * WE:(eb * 3 * WE) + (mi + 1) * WE] = \
                vm[:, eb * WE:(eb + 1) * WE]
    v8 = _q8((vc * V_SC).reshape(GD, 2, 128, 3 * D).transpose(0, 2, 1, 3))
    v8 = np.ascontiguousarray(v8)
    v0b = np.ascontiguousarray(v0.astype(ml_dtypes.bfloat16))
    adjT = np.ascontiguousarray(adj.T)
    eye = np.eye(128, dtype=np.float32)

    in_maps = []
    for c in range(N_CORES):
        x0T = inputs[c * BL:(c + 1) * BL].reshape(BN, D).T  # [D, BN]
        x8 = _q8(x0T.reshape(GD, 2, 128, BN).transpose(0, 2, 1, 3))
        in_maps.append({
            "x8": np.ascontiguousarray(x8),
            "xbf": np.ascontiguousarray(x0T.astype(ml_dtypes.bfloat16)),
            "v8": v8,
            "v0b": v0b,
            "adj": adj,
            "adjt": adjT,
            "bias": biases,
            "eye": eye,
        })
    return in_maps


def kernel(inputs, adj, weights, biases):
    nc = _get_program()
    in_maps = make_in_maps(inputs, adj, weights, biases)
    res = run_bass_kernel_spmd(nc, in_maps, list(range(N_CORES)))
    out = np.concatenate(
        [res.results[c]["out"].reshape(BL, N, D) for c in range(N_CORES)],
        axis=0)
    return out
